# revision 47
# baseline (speedup 1.0000x reference)
"""DeltaTokenShift Trainium2 kernel (Bass/Tile, 8 NeuronCores via axon).

Computation (per batch b):
    erase = sigmoid(x @ We + be) ; write = sigmoid(x @ Ww + bw)
    s_t = s_{t-1} * (1 - erase_t) + write_t * x_t   (scan over L, per channel)
    out[:, t, :] = s_t

Sharding: 8 cores = 4 batches x 2 halves of the 1024-channel dim. Each core
contracts over all 1024 input channels and produces its 512 output channels.
The channel dim is rotated per-half so the core's own 512 channels occupy
contraction rows 0..511 (a consistent permutation of the contraction dim
leaves the matmul result unchanged, and k-tiles m=0..3 then directly provide
the x operand of the w*x term).

Host-side layout packing (layout-only; all compute stays on device):
  x_r  [128, NCHUNK*KT*LC] bf16: x_r[p, (c*KT+k)*LC+l] = x[b][c*LC+l, perm[k*128+p]]
       -> one 1MB DMA per 512-token chunk, 8KB contiguous per partition.
  w_r  [128, KT*ESH] bf16 per gate: w_r[p, k*ESH+e] = W[perm[k*128+p], e0+e]
       -> lhsT slice for (k,m) is w_r[:, k*ESH+m*128 : k*ESH+(m+1)*128].
  The kernel writes its output transposed ([512 channels, 4096 tokens] f32);
  the host transposes back when reassembling the full [B, L, D] array.

Per chunk the PE runs only the 64 gate matmuls (bf16 lhsT/rhs, fp32 PSUM
accumulate), ACT applies sigmoid straight from PSUM (erase uses scale=-1,
bias=-be => 1-sigmoid), GpSimd forms b = write * x, DVE runs
tensor_tensor_scan(a, b) chained across chunks via initial=prev[:, -1:].
DMA queues: x chunks on the sync HWDGE ring; weights then outputs on the
scalar HWDGE ring, so input and output streams never share a ring.
"""

import sys

sys.path.insert(0, "/opt/trn_rl_repo")

import numpy as np
import ml_dtypes
import concourse.bacc as bacc
import concourse.mybir as mybir
from concourse.tile import TileContext
from concourse.bass_utils import run_bass_kernel_spmd

B, L = 4, 4096

F32 = mybir.dt.float32
BF16 = mybir.dt.bfloat16

P = 128
DIN = 1024
ESH = 512
KT = DIN // P  # 8 contraction k-tiles
MT = ESH // P  # 4 output-channel groups per core
LC = 512       # tokens per chunk
NCHUNK = L // LC
XW = KT * LC   # columns of one packed x chunk


def _build_kernel():
    nc = bacc.Bacc("TRN2", target_bir_lowering=False)

    xr = nc.dram_tensor("xr", [P, NCHUNK * XW], BF16, kind="ExternalInput")
    wer = nc.dram_tensor("wer", [P, KT * ESH], BF16, kind="ExternalInput")
    wwr = nc.dram_tensor("wwr", [P, KT * ESH], BF16, kind="ExternalInput")
    # consts[:, m] = -erase_bias group m ; consts[:, MT+m] = +write_bias
    # group m ; consts[:, 2*MT+m] = initial state group m
    consts = nc.dram_tensor("consts", [P, 3 * MT], F32, kind="ExternalInput")
    outt = nc.dram_tensor("outt", [ESH, L], F32, kind="ExternalOutput")

    with TileContext(nc) as tc:
        with (
            tc.tile_pool(name="const", bufs=1) as constp,
            tc.tile_pool(name="wsb", bufs=1) as wsb,
            tc.tile_pool(name="xsb", bufs=3) as xsb,
            tc.tile_pool(name="gate", bufs=6) as gatep,
            tc.tile_pool(name="bmul", bufs=6) as bmulp,
            tc.tile_pool(name="scan", bufs=3) as scanp,
            tc.tile_pool(name="ps_mm", bufs=6, space="PSUM") as ps_mm,
        ):
            # Chunk 0 computes all four erase groups before any write group,
            # so only `we` (1MB) gates the PE start; `ww` streams in behind
            # it. Weights ride the scalar ring in PE consumption order,
            # quartered for fine-grained unblocking; x chunk 0 rides the
            # sync ring in parallel, k-tile 0 first.
            w_sb = []
            for gi, wt in enumerate((wer, wwr)):
                t = wsb.tile([P, KT * ESH], BF16, tag=f"w{gi}")
                w_sb.append(t)
            WQ = KT * ESH // 4
            nc.scalar.dma_start(w_sb[0][:, :WQ // 2], wer[:, :WQ // 2])
            nc.scalar.dma_start(w_sb[0][:, WQ // 2:WQ], wer[:, WQ // 2:WQ])
            for q in range(1, 4):
                nc.scalar.dma_start(w_sb[0][:, q * WQ:(q + 1) * WQ],
                                    wer[:, q * WQ:(q + 1) * WQ])
            x0 = []
            const_sb = constp.tile([P, 3 * MT], F32, tag="consts")
            for k in range(KT):
                t = xsb.tile([P, LC], BF16, tag=f"x{k}")
                nc.sync.dma_start(t[:], xr[:, k * LC:(k + 1) * LC])
                x0.append(t)
                if k == 3:
                    # After k3: early enough for the first sigmoid
                    # (~11.5us), without delaying x0 k1-k3 dispatches that
                    # pace the first erase m-group.
                    nc.sync.dma_start(const_sb[:], consts[:])
            # Write-gate weights ride the sync ring behind x0: each ring
            # then preloads ~1.5MB instead of scalar carrying all 2MB, so
            # both gates' weights land before the PE needs them.
            for q in range(4):
                nc.sync.dma_start(w_sb[1][:, q * WQ:(q + 1) * WQ],
                                  wwr[:, q * WQ:(q + 1) * WQ])

            prev_s = [None] * MT
            # Out-DMA dispatches ride the sync engine's queue (which only
            # dispatches x prefetches otherwise). Each waits on its scan's
            # semaphore, and engines execute their queues in order —
            # dispatching immediately would park the queue head on that
            # wait. Holding each dispatch back three m-groups lets the scan
            # finish first, so the head never blocks.
            pending_out = []

            def flush_out(limit):
                while len(pending_out) > limit:
                    dst, src = pending_out.pop(0)
                    nc.sync.dma_start(dst, src)

            def mm_gate(gi, m, dst, xc):
                for k in range(KT):
                    nc.tensor.matmul(
                        dst[:],
                        w_sb[gi][:, k * ESH + m * P: k * ESH + (m + 1) * P],
                        xc[k][:],
                        start=(k == 0), stop=(k == KT - 1),
                    )

            def sig(dst, src, m, gi):
                nc.scalar.activation(
                    dst[:], src[:],
                    mybir.ActivationFunctionType.Sigmoid,
                    bias=const_sb[:, gi * MT + m:gi * MT + m + 1],
                    scale=-1.0 if gi == 0 else 1.0,
                )

            def wxscan(c, m, a_t, w_t, xc):
                b_t = bmulp.tile([P, LC], F32, tag="b")
                # GpSimd is otherwise idle; fully parallel with DVE, and
                # all operands + out are SBUF (P2-safe).
                nc.gpsimd.tensor_tensor(
                    b_t[:], w_t[:], xc[m][:], op=mybir.AluOpType.mult)
                s_t = scanp.tile([P, LC], F32, tag=f"s{m}")
                init = const_sb[:, 2 * MT + m:2 * MT + m + 1] if c == 0 \
                    else prev_s[m][:, LC - 1:LC]
                nc.vector.tensor_tensor_scan(
                    s_t[:], a_t[:], b_t[:], init,
                    op0=mybir.AluOpType.mult, op1=mybir.AluOpType.add,
                )
                prev_s[m] = s_t
                pending_out.append(
                    (outt[m * P:(m + 1) * P, c * LC:(c + 1) * LC], s_t[:]))

            # Chunk 0: all erase groups first (only `we` needed), then the
            # write groups as `ww` lands.
            a0 = []
            for m in range(MT):
                pe = ps_mm.tile([P, LC], F32, tag="psmm")
                mm_gate(0, m, pe, x0)
                a_t = gatep.tile([P, LC], F32, tag="a")
                sig(a_t, pe, m, 0)
                a0.append(a_t)
            for m in range(MT):
                pw = ps_mm.tile([P, LC], F32, tag="psmm")
                mm_gate(1, m, pw, x0)
                w_t = gatep.tile([P, LC], F32, tag="w")
                sig(w_t, pw, m, 1)
                wxscan(0, m, a0[m], w_t, x0)

            for c in range(1, NCHUNK - 1):
                xc = []
                for k in range(KT):
                    t = xsb.tile([P, LC], BF16, tag=f"x{k}")
                    nc.sync.dma_start(
                        t[:], xr[:, c * XW + k * LC:c * XW + (k + 1) * LC])
                    xc.append(t)

                for m in range(MT):
                    flush_out(2)
                    pe = ps_mm.tile([P, LC], F32, tag="psmm")
                    mm_gate(0, m, pe, xc)
                    a_t = gatep.tile([P, LC], F32, tag="a")
                    sig(a_t, pe, m, 0)

                    pw = ps_mm.tile([P, LC], F32, tag="psmm")
                    mm_gate(1, m, pw, xc)
                    w_t = gatep.tile([P, LC], F32, tag="w")
                    sig(w_t, pw, m, 1)

                    wxscan(c, m, a_t, w_t, xc)

            # Last chunk mirrors chunk 0: all write groups first, erase
            # groups last — the final dependency chain behind the last
            # matmul is then just sigmoid -> scan (the w*x mults are long
            # done), shortening the kernel tail.
            c = NCHUNK - 1
            xc = []
            for k in range(KT):
                t = xsb.tile([P, LC], BF16, tag=f"x{k}")
                nc.sync.dma_start(
                    t[:], xr[:, c * XW + k * LC:c * XW + (k + 1) * LC])
                xc.append(t)
            wl = []
            for m in range(MT):
                flush_out(0)
                pw = ps_mm.tile([P, LC], F32, tag="psmm")
                mm_gate(1, m, pw, xc)
                w_t = gatep.tile([P, LC], F32, tag="w")
                sig(w_t, pw, m, 1)
                wl.append(w_t)
            for m in range(MT):
                pe = ps_mm.tile([P, LC], F32, tag="psmm")
                mm_gate(0, m, pe, xc)
                a_t = gatep.tile([P, LC], F32, tag="a")
                sig(a_t, pe, m, 0)
                wxscan(c, m, a_t, wl[m], xc)
                # Tail: nothing rides either DMA queue after this point, so
                # head-blocking on the scan semaphore is free — dispatch
                # eagerly so each transfer overlaps the next group's work.
                flush_out(0)

    nc.finalize()
    return nc


_cached_nc = None


def _shard_inputs(x, state, erase_kernel, erase_bias, write_kernel, write_bias):
    maps = []
    for core in range(8):
        b, h = divmod(core, 2)
        e0 = h * ESH
        if h == 1:
            perm = np.concatenate([np.arange(ESH, DIN), np.arange(ESH)])
        else:
            perm = np.arange(DIN)
        xt = x[b].T[perm]  # [DIN, L]
        xrb = np.ascontiguousarray(
            xt.reshape(KT, P, NCHUNK, LC).transpose(1, 2, 0, 3).reshape(P, -1),
            dtype=ml_dtypes.bfloat16)
        web = erase_kernel[perm][:, e0:e0 + ESH]
        wwb = write_kernel[perm][:, e0:e0 + ESH]
        werb = np.ascontiguousarray(
            web.reshape(KT, P, ESH).transpose(1, 0, 2).reshape(P, -1),
            dtype=ml_dtypes.bfloat16)
        wwrb = np.ascontiguousarray(
            wwb.reshape(KT, P, ESH).transpose(1, 0, 2).reshape(P, -1),
            dtype=ml_dtypes.bfloat16)
        ben = (-erase_bias[e0:e0 + ESH]).reshape(MT, P).T
        bwp = write_bias[e0:e0 + ESH].reshape(MT, P).T
        stp = state[b, e0:e0 + ESH].reshape(MT, P).T
        maps.append({
            "xr": xrb,
            "wer": werb,
            "wwr": wwrb,
            "consts": np.ascontiguousarray(
                np.concatenate([ben, bwp, stp], axis=1), dtype=np.float32),
        })
    return maps


def kernel(x, state, erase_kernel, erase_bias, write_kernel, write_bias):
    global _cached_nc
    x = np.asarray(x, np.float32)
    state = np.asarray(state, np.float32)
    erase_kernel = np.asarray(erase_kernel, np.float32)
    erase_bias = np.asarray(erase_bias, np.float32)
    write_kernel = np.asarray(write_kernel, np.float32)
    write_bias = np.asarray(write_bias, np.float32)

    if _cached_nc is None:
        _cached_nc = _build_kernel()
    maps = _shard_inputs(x, state, erase_kernel, erase_bias,
                         write_kernel, write_bias)
    res = run_bass_kernel_spmd(_cached_nc, maps, core_ids=list(range(8)))
    full = np.empty((B, L, DIN), np.float32)
    for core in range(8):
        b, h = divmod(core, 2)
        full[b, :, h * ESH:(h + 1) * ESH] = res.results[core]["outt"].T
    return full


# revision 48
# speedup vs baseline: 1.0017x; 1.0017x over previous
"""DeltaTokenShift Trainium2 kernel (Bass/Tile, 8 NeuronCores via axon).

Computation (per batch b):
    erase = sigmoid(x @ We + be) ; write = sigmoid(x @ Ww + bw)
    s_t = s_{t-1} * (1 - erase_t) + write_t * x_t   (scan over L, per channel)
    out[:, t, :] = s_t

Sharding: 8 cores = 4 batches x 2 halves of the 1024-channel dim. Each core
contracts over all 1024 input channels and produces its 512 output channels.
The channel dim is rotated per-half so the core's own 512 channels occupy
contraction rows 0..511 (a consistent permutation of the contraction dim
leaves the matmul result unchanged, and k-tiles m=0..3 then directly provide
the x operand of the w*x term).

Host-side layout packing (layout-only; all compute stays on device):
  x_r  [128, NCHUNK*KT*LC] bf16: x_r[p, (c*KT+k)*LC+l] = x[b][c*LC+l, perm[k*128+p]]
       -> one 1MB DMA per 512-token chunk, 8KB contiguous per partition.
  w_r  [128, KT*ESH] bf16 per gate: w_r[p, k*ESH+e] = W[perm[k*128+p], e0+e]
       -> lhsT slice for (k,m) is w_r[:, k*ESH+m*128 : k*ESH+(m+1)*128].
  The kernel writes its output transposed ([512 channels, 4096 tokens] f32);
  the host transposes back when reassembling the full [B, L, D] array.

Per chunk the PE runs only the 64 gate matmuls (bf16 lhsT/rhs, fp32 PSUM
accumulate), ACT applies sigmoid straight from PSUM (erase uses scale=-1,
bias=-be => 1-sigmoid), GpSimd forms b = write * x, DVE runs
tensor_tensor_scan(a, b) chained across chunks via initial=prev[:, -1:].
DMA queues: x chunks on the sync HWDGE ring; weights then outputs on the
scalar HWDGE ring, so input and output streams never share a ring.
"""

import sys

sys.path.insert(0, "/opt/trn_rl_repo")

import numpy as np
import ml_dtypes
import concourse.bacc as bacc
import concourse.mybir as mybir
from concourse.tile import TileContext
from concourse.bass_utils import run_bass_kernel_spmd

B, L = 4, 4096

F32 = mybir.dt.float32
BF16 = mybir.dt.bfloat16

P = 128
DIN = 1024
ESH = 512
KT = DIN // P  # 8 contraction k-tiles
MT = ESH // P  # 4 output-channel groups per core
LC = 512       # tokens per chunk
NCHUNK = L // LC
XW = KT * LC   # columns of one packed x chunk


def _build_kernel():
    nc = bacc.Bacc("TRN2", target_bir_lowering=False)

    xr = nc.dram_tensor("xr", [P, NCHUNK * XW], BF16, kind="ExternalInput")
    wer = nc.dram_tensor("wer", [P, KT * ESH], BF16, kind="ExternalInput")
    wwr = nc.dram_tensor("wwr", [P, KT * ESH], BF16, kind="ExternalInput")
    # consts[:, m] = -erase_bias group m ; consts[:, MT+m] = +write_bias
    # group m ; consts[:, 2*MT+m] = initial state group m
    consts = nc.dram_tensor("consts", [P, 3 * MT], F32, kind="ExternalInput")
    outt = nc.dram_tensor("outt", [ESH, L], F32, kind="ExternalOutput")

    with TileContext(nc) as tc:
        with (
            tc.tile_pool(name="const", bufs=1) as constp,
            tc.tile_pool(name="wsb", bufs=1) as wsb,
            tc.tile_pool(name="xsb", bufs=3) as xsb,
            tc.tile_pool(name="gate", bufs=6) as gatep,
            tc.tile_pool(name="bmul", bufs=6) as bmulp,
            tc.tile_pool(name="scan", bufs=3) as scanp,
            tc.tile_pool(name="ps_mm", bufs=6, space="PSUM") as ps_mm,
        ):
            # Chunk 0 computes all four erase groups before any write group,
            # so only `we` (1MB) gates the PE start; `ww` streams in behind
            # it. Weights ride the scalar ring in PE consumption order,
            # quartered for fine-grained unblocking; x chunk 0 rides the
            # sync ring in parallel, k-tile 0 first.
            w_sb = []
            for gi, wt in enumerate((wer, wwr)):
                t = wsb.tile([P, KT * ESH], BF16, tag=f"w{gi}")
                w_sb.append(t)
            WQ = KT * ESH // 4
            nc.scalar.dma_start(w_sb[0][:, :WQ // 2], wer[:, :WQ // 2])
            nc.scalar.dma_start(w_sb[0][:, WQ // 2:WQ], wer[:, WQ // 2:WQ])
            for q in range(1, 4):
                nc.scalar.dma_start(w_sb[0][:, q * WQ:(q + 1) * WQ],
                                    wer[:, q * WQ:(q + 1) * WQ])
            x0 = []
            const_sb = constp.tile([P, 3 * MT], F32, tag="consts")
            for k in range(KT):
                t = xsb.tile([P, LC], BF16, tag=f"x{k}")
                nc.sync.dma_start(t[:], xr[:, k * LC:(k + 1) * LC])
                x0.append(t)
                if k == 0:
                    nc.sync.dma_start(const_sb[:], consts[:])
            # Write-gate weights ride the sync ring behind x0: each ring
            # then preloads ~1.5MB instead of scalar carrying all 2MB, so
            # both gates' weights land before the PE needs them.
            for q in range(4):
                nc.sync.dma_start(w_sb[1][:, q * WQ:(q + 1) * WQ],
                                  wwr[:, q * WQ:(q + 1) * WQ])

            prev_s = [None] * MT
            # Out-DMA dispatches ride the sync engine's queue (which only
            # dispatches x prefetches otherwise). Each waits on its scan's
            # semaphore, and engines execute their queues in order —
            # dispatching immediately would park the queue head on that
            # wait. Holding each dispatch back three m-groups lets the scan
            # finish first, so the head never blocks.
            pending_out = []

            def flush_out(limit):
                while len(pending_out) > limit:
                    dst, src = pending_out.pop(0)
                    nc.sync.dma_start(dst, src)

            def mm_gate(gi, m, dst, xc):
                for k in range(KT):
                    nc.tensor.matmul(
                        dst[:],
                        w_sb[gi][:, k * ESH + m * P: k * ESH + (m + 1) * P],
                        xc[k][:],
                        start=(k == 0), stop=(k == KT - 1),
                    )

            def sig(dst, src, m, gi):
                nc.scalar.activation(
                    dst[:], src[:],
                    mybir.ActivationFunctionType.Sigmoid,
                    bias=const_sb[:, gi * MT + m:gi * MT + m + 1],
                    scale=-1.0 if gi == 0 else 1.0,
                )

            def wxscan(c, m, a_t, w_t, xc):
                b_t = bmulp.tile([P, LC], F32, tag="b")
                # GpSimd is otherwise idle; fully parallel with DVE, and
                # all operands + out are SBUF (P2-safe).
                nc.gpsimd.tensor_tensor(
                    b_t[:], w_t[:], xc[m][:], op=mybir.AluOpType.mult)
                s_t = scanp.tile([P, LC], F32, tag=f"s{m}")
                init = const_sb[:, 2 * MT + m:2 * MT + m + 1] if c == 0 \
                    else prev_s[m][:, LC - 1:LC]
                nc.vector.tensor_tensor_scan(
                    s_t[:], a_t[:], b_t[:], init,
                    op0=mybir.AluOpType.mult, op1=mybir.AluOpType.add,
                )
                prev_s[m] = s_t
                pending_out.append(
                    (outt[m * P:(m + 1) * P, c * LC:(c + 1) * LC], s_t[:]))

            # Chunk 0: all erase groups first (only `we` needed), then the
            # write groups as `ww` lands.
            a0 = []
            for m in range(MT):
                pe = ps_mm.tile([P, LC], F32, tag="psmm")
                mm_gate(0, m, pe, x0)
                a_t = gatep.tile([P, LC], F32, tag="a")
                sig(a_t, pe, m, 0)
                a0.append(a_t)
            for m in range(MT):
                pw = ps_mm.tile([P, LC], F32, tag="psmm")
                mm_gate(1, m, pw, x0)
                w_t = gatep.tile([P, LC], F32, tag="w")
                sig(w_t, pw, m, 1)
                wxscan(0, m, a0[m], w_t, x0)

            for c in range(1, NCHUNK - 1):
                xc = []
                for k in range(KT):
                    t = xsb.tile([P, LC], BF16, tag=f"x{k}")
                    nc.sync.dma_start(
                        t[:], xr[:, c * XW + k * LC:c * XW + (k + 1) * LC])
                    xc.append(t)

                for m in range(MT):
                    flush_out(2)
                    pe = ps_mm.tile([P, LC], F32, tag="psmm")
                    mm_gate(0, m, pe, xc)
                    a_t = gatep.tile([P, LC], F32, tag="a")
                    sig(a_t, pe, m, 0)

                    pw = ps_mm.tile([P, LC], F32, tag="psmm")
                    mm_gate(1, m, pw, xc)
                    w_t = gatep.tile([P, LC], F32, tag="w")
                    sig(w_t, pw, m, 1)

                    wxscan(c, m, a_t, w_t, xc)

            # Last chunk mirrors chunk 0: all write groups first, erase
            # groups last — the final dependency chain behind the last
            # matmul is then just sigmoid -> scan (the w*x mults are long
            # done), shortening the kernel tail.
            c = NCHUNK - 1
            xc = []
            for k in range(KT):
                t = xsb.tile([P, LC], BF16, tag=f"x{k}")
                nc.sync.dma_start(
                    t[:], xr[:, c * XW + k * LC:c * XW + (k + 1) * LC])
                xc.append(t)
            wl = []
            for m in range(MT):
                flush_out(0)
                pw = ps_mm.tile([P, LC], F32, tag="psmm")
                mm_gate(1, m, pw, xc)
                w_t = gatep.tile([P, LC], F32, tag="w")
                sig(w_t, pw, m, 1)
                wl.append(w_t)
            for m in range(MT):
                pe = ps_mm.tile([P, LC], F32, tag="psmm")
                mm_gate(0, m, pe, xc)
                a_t = gatep.tile([P, LC], F32, tag="a")
                sig(a_t, pe, m, 0)
                wxscan(c, m, a_t, wl[m], xc)
                # Tail: nothing rides either DMA queue after this point, so
                # head-blocking on the scan semaphore is free — dispatch
                # eagerly so each transfer overlaps the next group's work.
                flush_out(0)

    nc.finalize()
    return nc


_cached_nc = None


def _shard_inputs(x, state, erase_kernel, erase_bias, write_kernel, write_bias):
    maps = []
    for core in range(8):
        b, h = divmod(core, 2)
        e0 = h * ESH
        if h == 1:
            perm = np.concatenate([np.arange(ESH, DIN), np.arange(ESH)])
        else:
            perm = np.arange(DIN)
        xt = x[b].T[perm]  # [DIN, L]
        xrb = np.ascontiguousarray(
            xt.reshape(KT, P, NCHUNK, LC).transpose(1, 2, 0, 3).reshape(P, -1),
            dtype=ml_dtypes.bfloat16)
        web = erase_kernel[perm][:, e0:e0 + ESH]
        wwb = write_kernel[perm][:, e0:e0 + ESH]
        werb = np.ascontiguousarray(
            web.reshape(KT, P, ESH).transpose(1, 0, 2).reshape(P, -1),
            dtype=ml_dtypes.bfloat16)
        wwrb = np.ascontiguousarray(
            wwb.reshape(KT, P, ESH).transpose(1, 0, 2).reshape(P, -1),
            dtype=ml_dtypes.bfloat16)
        ben = (-erase_bias[e0:e0 + ESH]).reshape(MT, P).T
        bwp = write_bias[e0:e0 + ESH].reshape(MT, P).T
        stp = state[b, e0:e0 + ESH].reshape(MT, P).T
        maps.append({
            "xr": xrb,
            "wer": werb,
            "wwr": wwrb,
            "consts": np.ascontiguousarray(
                np.concatenate([ben, bwp, stp], axis=1), dtype=np.float32),
        })
    return maps


def kernel(x, state, erase_kernel, erase_bias, write_kernel, write_bias):
    global _cached_nc
    x = np.asarray(x, np.float32)
    state = np.asarray(state, np.float32)
    erase_kernel = np.asarray(erase_kernel, np.float32)
    erase_bias = np.asarray(erase_bias, np.float32)
    write_kernel = np.asarray(write_kernel, np.float32)
    write_bias = np.asarray(write_bias, np.float32)

    if _cached_nc is None:
        _cached_nc = _build_kernel()
    maps = _shard_inputs(x, state, erase_kernel, erase_bias,
                         write_kernel, write_bias)
    res = run_bass_kernel_spmd(_cached_nc, maps, core_ids=list(range(8)))
    full = np.empty((B, L, DIN), np.float32)
    for core in range(8):
        b, h = divmod(core, 2)
        full[b, :, h * ESH:(h + 1) * ESH] = res.results[core]["outt"].T
    return full
